# revision 35
# baseline (speedup 1.0000x reference)
"""Trainium2 Bass kernel for nn_MultiHeadHighLevelAllocator.

Math (reference):
    ue = MLP3(uav_feat)                            # (B,U,E)
    te = MLP3(task_feat)                           # (B,T,E)
    q  = ue[:,None,:,:] + head_q[None,:,None,:]    # (B,H,U,E)
    logits[b,h,u,t] = relu(q[b,h,u]@Wq + te[b,t]@Wk + fb1) @ fw2 + fb2

Key decomposition: by linearity of the projections,
    pre[b,h,u,t,:] = base[b,u,t,:] + hqP[h,:]
where base[b,u,t,:] = ue[b,u]@Wq + te[b,t]@Wk  (outer sum, H-independent)
and   hqP[h,:] = head_q[h]@Wq + fb1.

Per core (data parallel over B, 2 batches/core), work is organized into
4096-column "units" (8 per core; cols = (u-block, t) pairs for one (b,c)
d-chunk).  Scheme per unit:

  B: TensorE builds base in PSUM via two accumulating broadcast-AP
     matmuls per 512 cols (Wq @ ue-bcast + Wk @ te-bcast), ScalarE
     copies PSUM->SBUF fp16 (FD=1024), VectorE does head relus as
     half-slab FD=2048 fused add+max ops (4x mode, started after two
     copies to halve the chain latency).  The unit's last bank is built
     by ScalarE directly (FD=128 khP + qP-bias ops) in its slack time,
     trimming two PE matmuls per unit; khP/qP projections are emitted
     lazily per section inside the pipeline.  `act_heads` optionally
     moves whole heads to ScalarE (unused in the final plan).
  A: VectorE builds base in SBUF fp16 (FD=128 tensor_scalar per u) from
     khP/qP projections, then long ADD,MAX ops per head (unused in the
     final plan).

The reduction against fw2 uses masked-stationary matmuls: a (128x32)
fp16 stationary holding the fw2-chunk in column j writes the dot product
row to PSUM partition 32g+j of strip g's own bank.  Strips cycle
round-robin (g = (n+h)%4) so up to 4 matmuls stream concurrently in
distinct PE column groups.  Reduction matmuls for unit k are emitted
after unit k+1's fills so the PE never idles waiting on the drain chain
(idle >3.4us re-throttles the PE clock to 1.2GHz).

A dummy-matmul warmup stream runs during the input DMA so the PE HAM
clock gate is at 2.4GHz before the encoders start.

All per-core inputs are packed host-side into ONE (128, 1824) fp16
tensor (weights/features fp16; biases bitcast fp32) loaded by
partition-split DMAs.
"""
import os
import sys

for _p in ("/opt/trn_rl_repo", "/root/.axon_site/_ro/trn_rl_repo"):
    if os.path.isdir(_p) and _p not in sys.path:
        sys.path.insert(0, _p)

import numpy as np
import concourse.bass as bass
import concourse.mybir as mybir
from concourse import tile

B, U, T = 16, 64, 128
UAV_DIM, TASK_DIM = 32, 32
E, H, HID = 128, 4, 256
ENC_H = 128
NCORES = 8
BL = B // NCORES          # batches per core
NBLK = U // 4             # 16 u-blocks of 4 us -> N=512 columns each
f32, f16 = mybir.dt.float32, mybir.dt.float16
AF = mybir.ActivationFunctionType
ALU = mybir.AluOpType

# packed tensor layout, fp16 columns
_C_UAVT = 0           # (32p, 128)
_C_TASKT = 128        # (32p, 256)
_C_UW0 = 384          # (32p, 128)
_C_TW0 = 512          # (32p, 128)
_C_B32 = 640          # 30 fp16 cols = 15 fp32: encb(7: ub0 ub1 ub2 tb0
#                       tb1 tb2 fb2) + hqpb(8: col c*4+h)
_C_UW1 = 672          # (128, 128) each
_C_UW2 = 800
_C_TW1 = 928
_C_TW2 = 1056
_C_WQK = 1184         # (128, 512): Wq c0 | Wq c1 | Wk c0 | Wk c1
# wz: two 63-col segments; fw2 chunk c at col c*63+31. The (128x32)
# masked stationary with fw2 at column j is the window [c*63+31-j, +32).
_C_WZ = 1696
_C_TOTAL = 1824

# number of N=128 dummy warmup matmuls issued before/through the encoder
_N_WARM = 6

# per-(b,c) section plan: scheme for each of the two 4096-col units,
# with the set of heads drained by ScalarE directly from PSUM ('B' only)
_PLAN = {
    (0, 0): (("B", ()), ("B", ())),
    (0, 1): (("B", ()), ("B", ())),
    (1, 0): (("B", ()), ("B", ())),
    (1, 1): (("B", ()), ("B", ())),
}

_cache: dict = {}


def _split_multi_waits(nc):
    """Walrus in this toolchain rejects >1 sync wait per engine instruction
    ("Too many sync wait commands"). Hoist extra waits onto preceding
    same-engine NoOps — identical semantics on the in-order engine queues."""
    n_split = 0
    for func in nc.m.functions:
        for bb in func.blocks:
            new = []
            for ins in bb.instructions:
                si = ins.sync_info
                waits = list(si.on_wait) if (si and si.on_wait) else []
                if len(waits) > 1:
                    for k, w in enumerate(waits[:-1]):
                        nop = mybir.InstNoOp(name=f"{ins.name}_hw{k}", ins=[], outs=[])
                        nop.engine = ins.engine
                        nop.sync_info = mybir.SyncInfo(on_wait=[w], on_update=[])
                        new.append(nop)
                        n_split += 1
                    si.on_wait = [waits[-1]]
                new.append(ins)
            bb.instructions = new
    return n_split


def _strip_map(b, h, n):
    """Round-robin strip assignment: strip g cycles 0..3 as (n+h) varies,
    j encodes (b, h, n//4). Bijective (b,h,n) <-> (g,j)."""
    g = (n + h) % 4
    j = b * 16 + h * 4 + n // 4
    return g, j


def _build_nc():
    nc = bass.Bass()
    packed = nc.dram_tensor("packed", [128, _C_TOTAL], f16, kind="ExternalInput")
    out = nc.dram_tensor("out", [128, 512], f32, kind="ExternalOutput")

    with tile.TileContext(nc) as tc:
        with (
            tc.tile_pool(name="const", bufs=1) as constp,
            tc.tile_pool(name="persist", bufs=1) as persistp,
            tc.tile_pool(name="encw", bufs=2) as encwp,
        ):
            A = constp.tile([128, _C_TOTAL], f16, tag="all")
            # phase-ordered, partition-split loads: L1 inputs + biases
            # first so the encoders start as early as possible
            for p0 in (0, 64):
                nc.sync.dma_start(A[p0:p0 + 64, :_C_UW1],
                                  packed[p0:p0 + 64, :_C_UW1])
            for p0 in (0, 64):
                nc.sync.dma_start(A[p0:p0 + 64, _C_UW1:_C_WQK],
                                  packed[p0:p0 + 64, _C_UW1:_C_WQK])
            for p0 in (0, 64):
                nc.sync.dma_start(A[p0:p0 + 64, _C_WQK:],
                                  packed[p0:p0 + 64, _C_WQK:])
            bc32 = A[:, _C_B32:_C_B32 + 30].bitcast(f32)
            sb_wz = A[:, _C_WZ:_C_WZ + 126]
            sb_wqk16 = A[:, _C_WQK:_C_WQK + 512]

            enc_w = {
                "uw0": A[0:32, _C_UW0:_C_UW0 + 128],
                "tw0": A[0:32, _C_TW0:_C_TW0 + 128],
                "uw1": A[:, _C_UW1:_C_UW1 + 128],
                "uw2": A[:, _C_UW2:_C_UW2 + 128],
                "tw1": A[:, _C_TW1:_C_TW1 + 128],
                "tw2": A[:, _C_TW2:_C_TW2 + 128],
            }

            def encb_col(i):
                return bc32[:, i:i + 1]

            def hqp_col(c, h):
                return bc32[:, 7 + c * 4 + h:7 + c * 4 + h + 1]

            # ---- encoder phase: own PSUM pools, released before sections ----
            with (
                tc.tile_pool(name="warmp", bufs=1, space="PSUM") as warmp,
                tc.tile_pool(name="encp", bufs=4, space="PSUM") as psE,
            ):
                # PE warmup stream: runs on memset data with no DMA
                # dependency, keeping the PE busy through the input DMA so
                # HAM is at K=8/8 when real matmuls start.  Encoder matmuls
                # are interleaved into the stream so the encoder pipeline
                # overlaps the warmup instead of queueing behind it.
                wsrc = constp.tile([128, 8], f16, tag="wsrc")
                nc.vector.memset(wsrc[:], 0.0)
                wps = warmp.tile([8, 512], f32, tag="warm")
                wmov = wsrc[:, 0:1].broadcast_to([128, 128])

                def warm(n):
                    for _ in range(n):
                        nc.tensor.matmul(wps[:, :128], wsrc[:], wmov,
                                         start=True, stop=True)

                warm(_N_WARM)
                chains = {  # noqa: encoder matmuls interleave with warm()
                    "ue": [A[0:32, _C_UAVT:_C_UAVT + BL * U], BL * U,
                           ("uw0", "uw1", "uw2"), (0, 1, 2)],
                    "te": [A[0:32, _C_TASKT:_C_TASKT + BL * T], BL * T,
                           ("tw0", "tw1", "tw2"), (3, 4, 5)],
                }
                cur = {k: v[0] for k, v in chains.items()}
                for li in range(3):
                    pss = {}
                    for k, (x0, rows, wn, bc) in chains.items():
                        ps = psE.tile([128, 512], f32, tag="ep",
                                      name=f"ps{k}{li}")
                        nc.tensor.matmul(ps[:, :rows], enc_w[wn[li]], cur[k],
                                         start=True, stop=True)
                        pss[k] = ps
                    warm(2)  # keep the PE busy while ScalarE runs the relus
                    for k, (x0, rows, wn, bc) in chains.items():
                        if li < 2:
                            nxt = encwp.tile([128, rows], f16, tag=f"{k}h",
                                             name=f"{k}h{li}")
                            nc.scalar.activation(nxt[:], pss[k][:, :rows],
                                                 AF.Relu,
                                                 bias=encb_col(bc[li]),
                                                 scale=1.0)
                        else:
                            nxt = persistp.tile([128, rows], f16, tag=f"{k}T",
                                                name=f"{k}T")
                            nc.scalar.activation(nxt[:], pss[k][:, :rows],
                                                 AF.Identity,
                                                 bias=encb_col(bc[li]),
                                                 scale=1.0)
                        cur[k] = nxt[:]
                ueT, teT = cur["ue"], cur["te"]

                khPs, qPs = {}, {}

            # ---- section phase ----
            with (
                tc.tile_pool(name="bsbp", bufs=4) as bsbp,
                tc.tile_pool(name="relup", bufs=10) as relup,
                tc.tile_pool(name="outp", bufs=1) as outp,
                tc.tile_pool(name="psSp", bufs=2, space="PSUM") as psS,
                tc.tile_pool(name="lpp", bufs=1, space="PSUM") as psL,
            ):
                lp = [psL.tile([128, 512], f32, tag=f"lp{g}", name=f"lp{g}")
                      for g in range(4)]
                n_red = [0, 0, 0, 0]
                N_RED_PER_STRIP = BL * 2 * NBLK * H // 4

                def red_mm(b, c, h, n, rt_ap):
                    """One masked-stationary reduction matmul: 512 cols of
                    rt for (b,c,h,u-block n) -> partition 32g+j of lp[g]."""
                    g, j = _strip_map(b, h, n)
                    first = n_red[g] == 0
                    last = n_red[g] == N_RED_PER_STRIP - 1
                    n_red[g] += 1
                    nc.tensor.matmul(
                        lp[g][32 * g:32 * g + 32, :],
                        sb_wz[:, c * 63 + 31 - j:c * 63 + 63 - j],
                        rt_ap,
                        start=first, stop=last,
                        tile_position=(0, 32 * g),
                        skip_group_check=True)

                def pe_base_fill(b, c, n0, ps_tile):
                    """Fill a [128,1024] PSUM tile (2 banks) with base cols
                    for u-blocks n0, n0+1 via broadcast-AP matmuls.  Both
                    Wq matmuls are issued back-to-back (then both Wk) so
                    the second LDWEIGHTS of each pair is a near-free
                    same-stationary reload."""
                    wq = sb_wqk16[:, c * 128:(c + 1) * 128]
                    wk = sb_wqk16[:, 256 + c * 128:256 + (c + 1) * 128]
                    tem = teT[:, b * T:(b + 1) * T].unsqueeze(1) \
                        .broadcast_to([128, 4, T])
                    for dn in range(2):
                        n = n0 + dn
                        uem = ueT[:, b * U + 4 * n:b * U + 4 * n + 4] \
                            .unsqueeze(2).broadcast_to([128, 4, T])
                        dst = ps_tile[:, dn * 512:(dn + 1) * 512]
                        nc.tensor.matmul(dst, wq, uem, start=True, stop=False)
                    for dn in range(2):
                        dst = ps_tile[:, dn * 512:(dn + 1) * 512]
                        nc.tensor.matmul(dst, wk, tem, start=False, stop=True)

                def prelude(b, c):
                    """Lazy per-section khP/qP projections, emitted inside
                    the pipeline (PSUM borrowed from the fill pool)."""
                    if (b, c) in khPs:
                        return
                    pp = psS.tile([128, 1024], f32, tag="ps",
                                  name=f"pp{b}{c}")
                    nc.tensor.matmul(pp[:, :T],
                                     sb_wqk16[:, 256 + c * 128:256 + (c + 1) * 128],
                                     teT[:, b * T:(b + 1) * T],
                                     start=True, stop=True)
                    nc.tensor.matmul(pp[:, 512:512 + U],
                                     sb_wqk16[:, c * 128:(c + 1) * 128],
                                     ueT[:, b * U:(b + 1) * U],
                                     start=True, stop=True)
                    khP = persistp.tile([128, T], f16, tag=f"khP{b}{c}",
                                        name=f"khP{b}{c}")
                    nc.scalar.copy(khP[:], pp[:, :T])
                    qP = persistp.tile([128, U], f32, tag=f"qP{b}{c}",
                                       name=f"qP{b}{c}")
                    nc.scalar.copy(qP[:], pp[:, 512:512 + U])
                    khPs[(b, c)], qPs[(b, c)] = khP, qP

                def pe_base_fill_single(b, c, n, dst):
                    wq = sb_wqk16[:, c * 128:(c + 1) * 128]
                    wk = sb_wqk16[:, 256 + c * 128:256 + (c + 1) * 128]
                    tem = teT[:, b * T:(b + 1) * T].unsqueeze(1) \
                        .broadcast_to([128, 4, T])
                    uem = ueT[:, b * U + 4 * n:b * U + 4 * n + 4] \
                        .unsqueeze(2).broadcast_to([128, 4, T])
                    nc.tensor.matmul(dst, wq, uem, start=True, stop=False)
                    nc.tensor.matmul(dst, wk, tem, start=False, stop=True)

                def emit_unit(b, c, n0, scheme, act_heads, fine=False):
                    """Emit fills + drains for one 4096-col unit
                    (u-blocks n0..n0+7); returns the per-head rt tiles."""
                    if scheme == "A":
                        khP, qP = khPs[(b, c)], qPs[(b, c)]
                        bsb = bsbp.tile([128, 4096], f16, tag="bsb",
                                        name="bsb")
                        for dn in range(8):
                            for du in range(4):
                                u = 4 * (n0 + dn) + du
                                nc.vector.tensor_scalar(
                                    bsb[:, dn * 512 + du * 128:
                                        dn * 512 + (du + 1) * 128],
                                    khP[:], qP[:, u:u + 1], None, ALU.add)
                        rts = {}
                        for h in range(H):
                            rt = relup.tile([128, 4096], f16, tag="rt",
                                            name="rt")
                            nc.vector.tensor_scalar(
                                rt[:], bsb[:], hqp_col(c, h), 0.0,
                                ALU.add, ALU.max)
                            rts[h] = rt
                        return rts

                    assert scheme == "B"
                    dve_heads = [h for h in range(H) if h not in act_heads]
                    rts = {h: relup.tile([128, 4096], f16, tag="rt",
                                         name="rt") for h in range(H)}
                    bsb = None
                    if dve_heads:
                        bsb = bsbp.tile([128, 4096], f16, tag="bsb",
                                        name="bsb")
                    prelude(b, c)
                    khP, qP = khPs[(b, c)], qPs[(b, c)]
                    for f in range(4):
                        ps_tile = psS.tile([128, 1024], f32, tag="ps",
                                           name=f"ps{b}{c}{n0}{f}")
                        if f < 3:
                            pe_base_fill(b, c, n0 + 2 * f, ps_tile)
                            nc.scalar.copy(bsb[:, f * 1024:(f + 1) * 1024],
                                           ps_tile[:])
                        else:
                            # bank 6 via the PE; bank 7 is built by ScalarE
                            # (FD=128 khP+qP bias ops) in its slack time,
                            # trimming two PE matmuls per unit
                            pe_base_fill_single(b, c, n0 + 6,
                                                ps_tile[:, 0:512])
                            for du in range(4):
                                u = 4 * (n0 + 7) + du
                                nc.scalar.activation(
                                    bsb[:, 3584 + du * 128:
                                        3584 + (du + 1) * 128],
                                    khP[:], AF.Identity,
                                    bias=qP[:, u:u + 1], scale=1.0)
                            nc.scalar.copy(bsb[:, 3072:3584],
                                           ps_tile[:, 0:512])
                        # half-slab relus (FD=2048): halves the copy->relu
                        # chain latency so reduction matmuls never stall
                        if f in (1, 3):
                            lo = (f - 1) * 1024
                            for h in dve_heads:
                                nc.vector.tensor_scalar(
                                    rts[h][:, lo:lo + 2048],
                                    bsb[:, lo:lo + 2048], hqp_col(c, h), 0.0,
                                    ALU.add, ALU.max)
                    return rts

                units = []
                for b in range(BL):
                    for c in range(2):
                        for ui, (scheme, ah) in enumerate(_PLAN[(b, c)]):
                            units.append((b, c, 8 * ui, scheme, ah))

                pending = None  # (b, c, n0, rts) awaiting reduction
                for ui, (b, c, n0, scheme, ah) in enumerate(units):
                    rts = emit_unit(b, c, n0, scheme, ah,
                                    fine=(ui == len(units) - 1))
                    if pending is not None:
                        pb, pc, pn0, prts = pending
                        for h in range(H):
                            for dn in range(8):
                                red_mm(pb, pc, h, pn0 + dn,
                                       prts[h][:, dn * 512:(dn + 1) * 512])
                    pending = (b, c, n0, rts)
                pb, pc, pn0, prts = pending
                for h in range(H):
                    for dn in range(8):
                        red_mm(pb, pc, h, pn0 + dn,
                               prts[h][:, dn * 512:(dn + 1) * 512])

                # per-strip epilogue split across ScalarE and VectorE (both
                # idle at the tail), with a split DMA so each strip ships
                # as soon as it is ready
                sb_out = outp.tile([128, 512], f32, tag="sbout", name="sbout")
                for g in range(4):
                    if g % 2 == 0:
                        nc.scalar.activation(
                            sb_out[32 * g:32 * g + 32, :],
                            lp[g][32 * g:32 * g + 32, :],
                            AF.Identity, bias=bc32[32 * g:32 * g + 32, 6:7],
                            scale=1.0)
                    else:
                        nc.vector.tensor_scalar(
                            sb_out[32 * g:32 * g + 32, :],
                            lp[g][32 * g:32 * g + 32, :],
                            bc32[32 * g:32 * g + 32, 6:7],
                            None, ALU.add)
                    nc.sync.dma_start(out[32 * g:32 * g + 32, :],
                                      sb_out[32 * g:32 * g + 32, :])
    return nc


def _prep_inputs(uav_feat, task_feat, uw0, ub0, uw1, ub1, uw2, ub2,
                 tw0, tb0, tw1, tb1, tw2, tb2, head_q, fw1, fb1, fw2, fb2):
    f, f2 = np.float32, np.float16
    uav = np.asarray(uav_feat, f)
    task = np.asarray(task_feat, f)
    fw1 = np.asarray(fw1, f)
    fw2 = np.asarray(fw2, f)
    Wq, Wk = fw1[:E], fw1[E:]

    base = np.zeros((128, _C_TOTAL), f2)
    base[0:32, _C_UW0:_C_UW0 + 128] = np.asarray(uw0, f2)
    base[0:32, _C_TW0:_C_TW0 + 128] = np.asarray(tw0, f2)
    base[:, _C_UW1:_C_UW1 + 128] = np.asarray(uw1, f2)
    base[:, _C_UW2:_C_UW2 + 128] = np.asarray(uw2, f2)
    base[:, _C_TW1:_C_TW1 + 128] = np.asarray(tw1, f2)
    base[:, _C_TW2:_C_TW2 + 128] = np.asarray(tw2, f2)
    # fp32 bias region, bitcast into the fp16 tensor
    b32 = np.zeros((128, 15), f)
    for i, v in enumerate((ub0, ub1, ub2, tb0, tb1, tb2)):
        b32[:, i] = np.asarray(v, f)
    b32[:, 6] = np.asarray(fb2, f)[0]
    hq = np.asarray(head_q, f) @ Wq + np.asarray(fb1, f)  # (H, HID)
    for c in range(2):
        for h in range(H):
            b32[:, 7 + c * 4 + h] = hq[h, c * 128:(c + 1) * 128]
    base[:, _C_B32:_C_B32 + 30] = b32.view(f2).reshape(128, 30)
    base[:, _C_WQK:_C_WQK + 256] = Wq.astype(f2)
    base[:, _C_WQK + 256:_C_WQK + 512] = Wk.astype(f2)
    for c in range(2):
        base[:, _C_WZ + c * 63 + 31] = fw2[c * 128:(c + 1) * 128, 0].astype(f2)

    in_maps = []
    for k in range(NCORES):
        b0 = k * BL
        pk = base.copy()
        pk[0:32, _C_UAVT:_C_UAVT + BL * U] = \
            uav[b0:b0 + BL].reshape(BL * U, UAV_DIM).T.astype(f2)
        pk[0:32, _C_TASKT:_C_TASKT + BL * T] = \
            task[b0:b0 + BL].reshape(BL * T, TASK_DIM).T.astype(f2)
        in_maps.append({"packed": pk})
    return in_maps


def _gather(results):
    # out row p = 32*g + b*16 + h*4 + n4  with  n = 4*n4 + (g-h)%4,
    # cols = du*128 + t,  u = 4n + du.
    outs = []
    for k in range(NCORES):
        r = np.asarray(results[k]["out"], np.float32)  # (128, 512)
        o = np.empty((BL, H, U, T), np.float32)
        for g in range(4):
            for b in range(BL):
                for h in range(H):
                    for n4 in range(4):
                        p = 32 * g + b * 16 + h * 4 + n4
                        n = 4 * n4 + (g - h) % 4
                        row = r[p].reshape(4, T)      # (du, T)
                        o[b, h, 4 * n:4 * n + 4, :] = row
        outs.append(o)
    return np.concatenate(outs, axis=0)


def kernel(**inputs) -> np.ndarray:
    if "nc" not in _cache:
        _cache["nc"] = _build_nc()
    nc = _cache["nc"]
    in_maps = _prep_inputs(**inputs)
    if os.environ.get("BASS_KERNEL_SIM"):
        # CoreSim can't digest the hand-inserted wait-splitting NoOps; it
        # enforces the multi-wait semantics natively, so run unsplit.
        from concourse.bass_interp import CoreSim
        results = []
        for k in range(NCORES):
            sim = CoreSim(nc, require_nnan=False)
            for name, arr in in_maps[k].items():
                sim.tensor(name)[:] = arr
            sim.simulate()
            results.append({"out": np.array(sim.tensor("out"))})
    else:
        from concourse.bass_utils import run_bass_kernel_spmd
        if not _cache.get("split"):
            _split_multi_waits(nc)
            _cache["split"] = True
        results = run_bass_kernel_spmd(nc, in_maps, list(range(NCORES))).results
    return _gather(results)


# revision 36
# speedup vs baseline: 1.2010x; 1.2010x over previous
"""Trainium2 Bass kernel for nn_MultiHeadHighLevelAllocator.

Math (reference):
    ue = MLP3(uav_feat)                            # (B,U,E)
    te = MLP3(task_feat)                           # (B,T,E)
    q  = ue[:,None,:,:] + head_q[None,:,None,:]    # (B,H,U,E)
    logits[b,h,u,t] = relu(q[b,h,u]@Wq + te[b,t]@Wk + fb1) @ fw2 + fb2

Key decomposition: by linearity of the projections,
    pre[b,h,u,t,:] = base[b,u,t,:] + hqP[h,:]
where base[b,u,t,:] = ue[b,u]@Wq + te[b,t]@Wk  (outer sum, H-independent)
and   hqP[h,:] = head_q[h]@Wq + fb1.

Per core (data parallel over B, 2 batches/core), work is organized into
4096-column "units" (8 per core; cols = (u-block, t) pairs for one (b,c)
d-chunk).  Scheme per unit:

  B: TensorE builds base in PSUM via two accumulating broadcast-AP
     matmuls per 512 cols (Wq @ ue-bcast + Wk @ te-bcast), ScalarE
     copies PSUM->SBUF fp16 (FD=1024), VectorE does head relus as
     half-slab FD=2048 fused add+max ops (4x mode, started after two
     copies to halve the chain latency).  The unit's last bank is built
     by ScalarE directly (FD=128 khP + qP-bias ops) in its slack time,
     trimming two PE matmuls per unit; khP/qP projections are emitted
     lazily per section inside the pipeline.  `act_heads` optionally
     moves whole heads to ScalarE (unused in the final plan).
  A: VectorE builds base in SBUF fp16 (FD=128 tensor_scalar per u) from
     khP/qP projections, then long ADD,MAX ops per head (unused in the
     final plan).

The reduction against fw2 uses masked-stationary matmuls: a (128x32)
fp16 stationary holding the fw2-chunk in column j writes the dot product
row to PSUM partition 32g+j of strip g's own bank.  Strips cycle
round-robin (g = (n+h)%4) so up to 4 matmuls stream concurrently in
distinct PE column groups.  Reduction matmuls for unit k are emitted
after unit k+1's fills so the PE never idles waiting on the drain chain
(idle >3.4us re-throttles the PE clock to 1.2GHz).

A dummy-matmul warmup stream runs during the input DMA so the PE HAM
clock gate is at 2.4GHz before the encoders start.

All per-core inputs are packed host-side into ONE (128, 1824) fp16
tensor (weights/features fp16; biases bitcast fp32) loaded by
partition-split DMAs.
"""
import os
import sys

for _p in ("/opt/trn_rl_repo", "/root/.axon_site/_ro/trn_rl_repo"):
    if os.path.isdir(_p) and _p not in sys.path:
        sys.path.insert(0, _p)

import numpy as np
import concourse.bass as bass
import concourse.mybir as mybir
from concourse import tile

B, U, T = 16, 64, 128
UAV_DIM, TASK_DIM = 32, 32
E, H, HID = 128, 4, 256
ENC_H = 128
NCORES = 8
BL = B // NCORES          # batches per core
NBLK = U // 4             # 16 u-blocks of 4 us -> N=512 columns each
f32, f16 = mybir.dt.float32, mybir.dt.float16
AF = mybir.ActivationFunctionType
ALU = mybir.AluOpType

# packed tensor layout, fp16 columns
_C_UAVT = 0           # (32p, 128)
_C_TASKT = 128        # (32p, 256)
_C_UW0 = 384          # (32p, 128)
_C_TW0 = 512          # (32p, 128)
_C_B32 = 640          # 30 fp16 cols = 15 fp32: encb(7: ub0 ub1 ub2 tb0
#                       tb1 tb2 fb2) + hqpb(8: col c*4+h)
_C_UW1 = 672          # (128, 128) each
_C_UW2 = 800
_C_TW1 = 928
_C_TW2 = 1056
_C_WQK = 1184         # (128, 512): Wq c0 | Wq c1 | Wk c0 | Wk c1
# wz: two 63-col segments; fw2 chunk c at col c*63+31. The (128x32)
# masked stationary with fw2 at column j is the window [c*63+31-j, +32).
_C_WZ = 1696
_C_TOTAL = 1824

# number of N=128 dummy warmup matmuls issued before/through the encoder
_N_WARM = 16

# per-(b,c) section plan: scheme for each of the two 4096-col units,
# with the set of heads drained by ScalarE directly from PSUM ('B' only)
_PLAN = {
    (0, 0): (("B", ()), ("B", ())),
    (0, 1): (("B", ()), ("B", ())),
    (1, 0): (("B", ()), ("B", ())),
    (1, 1): (("B", ()), ("B", ())),
}

_cache: dict = {}


def _split_multi_waits(nc):
    """Walrus in this toolchain rejects >1 sync wait per engine instruction
    ("Too many sync wait commands"). Hoist extra waits onto preceding
    same-engine NoOps — identical semantics on the in-order engine queues."""
    n_split = 0
    for func in nc.m.functions:
        for bb in func.blocks:
            new = []
            for ins in bb.instructions:
                si = ins.sync_info
                waits = list(si.on_wait) if (si and si.on_wait) else []
                if len(waits) > 1:
                    for k, w in enumerate(waits[:-1]):
                        nop = mybir.InstNoOp(name=f"{ins.name}_hw{k}", ins=[], outs=[])
                        nop.engine = ins.engine
                        nop.sync_info = mybir.SyncInfo(on_wait=[w], on_update=[])
                        new.append(nop)
                        n_split += 1
                    si.on_wait = [waits[-1]]
                new.append(ins)
            bb.instructions = new
    return n_split


def _strip_map(b, h, n):
    """Round-robin strip assignment: strip g cycles 0..3 as (n+h) varies,
    j encodes (b, h, n//4). Bijective (b,h,n) <-> (g,j)."""
    g = (n + h) % 4
    j = b * 16 + h * 4 + n // 4
    return g, j


def _build_nc():
    nc = bass.Bass()
    packed = nc.dram_tensor("packed", [128, _C_TOTAL], f16, kind="ExternalInput")
    out = nc.dram_tensor("out", [128, 512], f32, kind="ExternalOutput")

    with tile.TileContext(nc) as tc:
        with (
            tc.tile_pool(name="const", bufs=1) as constp,
            tc.tile_pool(name="persist", bufs=1) as persistp,
            tc.tile_pool(name="encw", bufs=2) as encwp,
        ):
            A = constp.tile([128, _C_TOTAL], f16, tag="all")
            # phase-ordered, partition-split loads: L1 inputs + biases
            # first so the encoders start as early as possible
            for p0 in (0, 64):
                nc.sync.dma_start(A[p0:p0 + 64, :_C_UW1],
                                  packed[p0:p0 + 64, :_C_UW1])
            for p0 in (0, 64):
                nc.sync.dma_start(A[p0:p0 + 64, _C_UW1:_C_WQK],
                                  packed[p0:p0 + 64, _C_UW1:_C_WQK])
            for p0 in (0, 64):
                nc.sync.dma_start(A[p0:p0 + 64, _C_WQK:],
                                  packed[p0:p0 + 64, _C_WQK:])
            bc32 = A[:, _C_B32:_C_B32 + 30].bitcast(f32)
            sb_wz = A[:, _C_WZ:_C_WZ + 126]
            sb_wqk16 = A[:, _C_WQK:_C_WQK + 512]

            enc_w = {
                "uw0": A[0:32, _C_UW0:_C_UW0 + 128],
                "tw0": A[0:32, _C_TW0:_C_TW0 + 128],
                "uw1": A[:, _C_UW1:_C_UW1 + 128],
                "uw2": A[:, _C_UW2:_C_UW2 + 128],
                "tw1": A[:, _C_TW1:_C_TW1 + 128],
                "tw2": A[:, _C_TW2:_C_TW2 + 128],
            }

            def encb_col(i):
                return bc32[:, i:i + 1]

            def hqp_col(c, h):
                return bc32[:, 7 + c * 4 + h:7 + c * 4 + h + 1]

            # ---- encoder phase: own PSUM pools, released before sections ----
            with (
                tc.tile_pool(name="warmp", bufs=1, space="PSUM") as warmp,
                tc.tile_pool(name="encp", bufs=4, space="PSUM") as psE,
            ):
                # PE warmup stream: runs on memset data with no DMA
                # dependency, keeping the PE busy through the input DMA so
                # HAM is at K=8/8 when real matmuls start.  Encoder matmuls
                # are interleaved into the stream so the encoder pipeline
                # overlaps the warmup instead of queueing behind it.
                wsrc = constp.tile([128, 8], f16, tag="wsrc")
                nc.vector.memset(wsrc[:], 0.0)
                wps = warmp.tile([8, 512], f32, tag="warm")
                wmov = wsrc[:, 0:1].broadcast_to([128, 128])

                def warm(n):
                    for _ in range(n):
                        nc.tensor.matmul(wps[:, :128], wsrc[:], wmov,
                                         start=True, stop=True)

                warm(_N_WARM)
                chains = {  # noqa: encoder matmuls interleave with warm()
                    "ue": [A[0:32, _C_UAVT:_C_UAVT + BL * U], BL * U,
                           ("uw0", "uw1", "uw2"), (0, 1, 2)],
                    "te": [A[0:32, _C_TASKT:_C_TASKT + BL * T], BL * T,
                           ("tw0", "tw1", "tw2"), (3, 4, 5)],
                }
                cur = {k: v[0] for k, v in chains.items()}
                for li in range(3):
                    pss = {}
                    for k, (x0, rows, wn, bc) in chains.items():
                        ps = psE.tile([128, 512], f32, tag="ep",
                                      name=f"ps{k}{li}")
                        nc.tensor.matmul(ps[:, :rows], enc_w[wn[li]], cur[k],
                                         start=True, stop=True)
                        pss[k] = ps
                    warm(9)  # keep the PE busy while ScalarE runs the relus
                    for k, (x0, rows, wn, bc) in chains.items():
                        if li < 2:
                            nxt = encwp.tile([128, rows], f16, tag=f"{k}h",
                                             name=f"{k}h{li}")
                            nc.scalar.activation(nxt[:], pss[k][:, :rows],
                                                 AF.Relu,
                                                 bias=encb_col(bc[li]),
                                                 scale=1.0)
                        else:
                            nxt = persistp.tile([128, rows], f16, tag=f"{k}T",
                                                name=f"{k}T")
                            nc.scalar.activation(nxt[:], pss[k][:, :rows],
                                                 AF.Identity,
                                                 bias=encb_col(bc[li]),
                                                 scale=1.0)
                        cur[k] = nxt[:]
                ueT, teT = cur["ue"], cur["te"]

                khPs, qPs = {}, {}

            # ---- section phase ----
            with (
                tc.tile_pool(name="bsbp", bufs=4) as bsbp,
                tc.tile_pool(name="relup", bufs=10) as relup,
                tc.tile_pool(name="outp", bufs=1) as outp,
                tc.tile_pool(name="psSp", bufs=2, space="PSUM") as psS,
                tc.tile_pool(name="lpp", bufs=1, space="PSUM") as psL,
            ):
                lp = [psL.tile([128, 512], f32, tag=f"lp{g}", name=f"lp{g}")
                      for g in range(4)]
                n_red = [0, 0, 0, 0]
                N_RED_PER_STRIP = BL * 2 * NBLK * H // 4

                def red_mm(b, c, h, n, rt_ap):
                    """One masked-stationary reduction matmul: 512 cols of
                    rt for (b,c,h,u-block n) -> partition 32g+j of lp[g]."""
                    g, j = _strip_map(b, h, n)
                    first = n_red[g] == 0
                    last = n_red[g] == N_RED_PER_STRIP - 1
                    n_red[g] += 1
                    nc.tensor.matmul(
                        lp[g][32 * g:32 * g + 32, :],
                        sb_wz[:, c * 63 + 31 - j:c * 63 + 63 - j],
                        rt_ap,
                        start=first, stop=last,
                        tile_position=(0, 32 * g),
                        skip_group_check=True)

                def pe_base_fill(b, c, n0, ps_tile):
                    """Fill a [128,1024] PSUM tile (2 banks) with base cols
                    for u-blocks n0, n0+1 via broadcast-AP matmuls.  Both
                    Wq matmuls are issued back-to-back (then both Wk) so
                    the second LDWEIGHTS of each pair is a near-free
                    same-stationary reload."""
                    wq = sb_wqk16[:, c * 128:(c + 1) * 128]
                    wk = sb_wqk16[:, 256 + c * 128:256 + (c + 1) * 128]
                    tem = teT[:, b * T:(b + 1) * T].unsqueeze(1) \
                        .broadcast_to([128, 4, T])
                    for dn in range(2):
                        n = n0 + dn
                        uem = ueT[:, b * U + 4 * n:b * U + 4 * n + 4] \
                            .unsqueeze(2).broadcast_to([128, 4, T])
                        dst = ps_tile[:, dn * 512:(dn + 1) * 512]
                        nc.tensor.matmul(dst, wq, uem, start=True, stop=False)
                    for dn in range(2):
                        dst = ps_tile[:, dn * 512:(dn + 1) * 512]
                        nc.tensor.matmul(dst, wk, tem, start=False, stop=True)

                def prelude(b, c):
                    """Lazy per-section khP/qP projections, emitted inside
                    the pipeline (PSUM borrowed from the fill pool)."""
                    if (b, c) in khPs:
                        return
                    pp = psS.tile([128, 1024], f32, tag="ps",
                                  name=f"pp{b}{c}")
                    nc.tensor.matmul(pp[:, :T],
                                     sb_wqk16[:, 256 + c * 128:256 + (c + 1) * 128],
                                     teT[:, b * T:(b + 1) * T],
                                     start=True, stop=True)
                    nc.tensor.matmul(pp[:, 512:512 + U],
                                     sb_wqk16[:, c * 128:(c + 1) * 128],
                                     ueT[:, b * U:(b + 1) * U],
                                     start=True, stop=True)
                    khP = persistp.tile([128, T], f16, tag=f"khP{b}{c}",
                                        name=f"khP{b}{c}")
                    nc.scalar.copy(khP[:], pp[:, :T])
                    qP = persistp.tile([128, U], f32, tag=f"qP{b}{c}",
                                       name=f"qP{b}{c}")
                    nc.scalar.copy(qP[:], pp[:, 512:512 + U])
                    khPs[(b, c)], qPs[(b, c)] = khP, qP

                def pe_base_fill_single(b, c, n, dst):
                    wq = sb_wqk16[:, c * 128:(c + 1) * 128]
                    wk = sb_wqk16[:, 256 + c * 128:256 + (c + 1) * 128]
                    tem = teT[:, b * T:(b + 1) * T].unsqueeze(1) \
                        .broadcast_to([128, 4, T])
                    uem = ueT[:, b * U + 4 * n:b * U + 4 * n + 4] \
                        .unsqueeze(2).broadcast_to([128, 4, T])
                    nc.tensor.matmul(dst, wq, uem, start=True, stop=False)
                    nc.tensor.matmul(dst, wk, tem, start=False, stop=True)

                def emit_unit(b, c, n0, scheme, act_heads, fine=False):
                    """Emit fills + drains for one 4096-col unit
                    (u-blocks n0..n0+7); returns the per-head rt tiles."""
                    if scheme == "A":
                        khP, qP = khPs[(b, c)], qPs[(b, c)]
                        bsb = bsbp.tile([128, 4096], f16, tag="bsb",
                                        name="bsb")
                        for dn in range(8):
                            for du in range(4):
                                u = 4 * (n0 + dn) + du
                                nc.vector.tensor_scalar(
                                    bsb[:, dn * 512 + du * 128:
                                        dn * 512 + (du + 1) * 128],
                                    khP[:], qP[:, u:u + 1], None, ALU.add)
                        rts = {}
                        for h in range(H):
                            rt = relup.tile([128, 4096], f16, tag="rt",
                                            name="rt")
                            nc.vector.tensor_scalar(
                                rt[:], bsb[:], hqp_col(c, h), 0.0,
                                ALU.add, ALU.max)
                            rts[h] = rt
                        return rts

                    assert scheme == "B"
                    dve_heads = [h for h in range(H) if h not in act_heads]
                    rts = {h: relup.tile([128, 4096], f16, tag="rt",
                                         name="rt") for h in range(H)}
                    bsb = None
                    if dve_heads:
                        bsb = bsbp.tile([128, 4096], f16, tag="bsb",
                                        name="bsb")
                    prelude(b, c)
                    khP, qP = khPs[(b, c)], qPs[(b, c)]
                    for f in range(4):
                        ps_tile = psS.tile([128, 1024], f32, tag="ps",
                                           name=f"ps{b}{c}{n0}{f}")
                        if f < 3:
                            pe_base_fill(b, c, n0 + 2 * f, ps_tile)
                            nc.scalar.copy(bsb[:, f * 1024:(f + 1) * 1024],
                                           ps_tile[:])
                        else:
                            # bank 6 via the PE; bank 7 is built by ScalarE
                            # (FD=128 khP+qP bias ops) in its slack time,
                            # trimming two PE matmuls per unit
                            pe_base_fill_single(b, c, n0 + 6,
                                                ps_tile[:, 0:512])
                            for du in range(4):
                                u = 4 * (n0 + 7) + du
                                nc.scalar.activation(
                                    bsb[:, 3584 + du * 128:
                                        3584 + (du + 1) * 128],
                                    khP[:], AF.Identity,
                                    bias=qP[:, u:u + 1], scale=1.0)
                            nc.scalar.copy(bsb[:, 3072:3584],
                                           ps_tile[:, 0:512])
                        # half-slab relus (FD=2048): halves the copy->relu
                        # chain latency so reduction matmuls never stall
                        if f in (1, 3):
                            lo = (f - 1) * 1024
                            for h in dve_heads:
                                nc.vector.tensor_scalar(
                                    rts[h][:, lo:lo + 2048],
                                    bsb[:, lo:lo + 2048], hqp_col(c, h), 0.0,
                                    ALU.add, ALU.max)
                    return rts

                units = []
                for b in range(BL):
                    for c in range(2):
                        for ui, (scheme, ah) in enumerate(_PLAN[(b, c)]):
                            units.append((b, c, 8 * ui, scheme, ah))

                pending = None  # (b, c, n0, rts) awaiting reduction
                for ui, (b, c, n0, scheme, ah) in enumerate(units):
                    rts = emit_unit(b, c, n0, scheme, ah,
                                    fine=(ui == len(units) - 1))
                    if pending is not None:
                        pb, pc, pn0, prts = pending
                        for h in range(H):
                            for dn in range(8):
                                red_mm(pb, pc, h, pn0 + dn,
                                       prts[h][:, dn * 512:(dn + 1) * 512])
                    pending = (b, c, n0, rts)
                pb, pc, pn0, prts = pending
                for h in range(H):
                    for dn in range(8):
                        red_mm(pb, pc, h, pn0 + dn,
                               prts[h][:, dn * 512:(dn + 1) * 512])

                # per-strip epilogue split across ScalarE and VectorE (both
                # idle at the tail), with a split DMA so each strip ships
                # as soon as it is ready
                sb_out = outp.tile([128, 512], f32, tag="sbout", name="sbout")
                for g in range(4):
                    if g % 2 == 0:
                        nc.scalar.activation(
                            sb_out[32 * g:32 * g + 32, :],
                            lp[g][32 * g:32 * g + 32, :],
                            AF.Identity, bias=bc32[32 * g:32 * g + 32, 6:7],
                            scale=1.0)
                    else:
                        nc.vector.tensor_scalar(
                            sb_out[32 * g:32 * g + 32, :],
                            lp[g][32 * g:32 * g + 32, :],
                            bc32[32 * g:32 * g + 32, 6:7],
                            None, ALU.add)
                    nc.sync.dma_start(out[32 * g:32 * g + 32, :],
                                      sb_out[32 * g:32 * g + 32, :])
    return nc


def _prep_inputs(uav_feat, task_feat, uw0, ub0, uw1, ub1, uw2, ub2,
                 tw0, tb0, tw1, tb1, tw2, tb2, head_q, fw1, fb1, fw2, fb2):
    f, f2 = np.float32, np.float16
    uav = np.asarray(uav_feat, f)
    task = np.asarray(task_feat, f)
    fw1 = np.asarray(fw1, f)
    fw2 = np.asarray(fw2, f)
    Wq, Wk = fw1[:E], fw1[E:]

    base = np.zeros((128, _C_TOTAL), f2)
    base[0:32, _C_UW0:_C_UW0 + 128] = np.asarray(uw0, f2)
    base[0:32, _C_TW0:_C_TW0 + 128] = np.asarray(tw0, f2)
    base[:, _C_UW1:_C_UW1 + 128] = np.asarray(uw1, f2)
    base[:, _C_UW2:_C_UW2 + 128] = np.asarray(uw2, f2)
    base[:, _C_TW1:_C_TW1 + 128] = np.asarray(tw1, f2)
    base[:, _C_TW2:_C_TW2 + 128] = np.asarray(tw2, f2)
    # fp32 bias region, bitcast into the fp16 tensor
    b32 = np.zeros((128, 15), f)
    for i, v in enumerate((ub0, ub1, ub2, tb0, tb1, tb2)):
        b32[:, i] = np.asarray(v, f)
    b32[:, 6] = np.asarray(fb2, f)[0]
    hq = np.asarray(head_q, f) @ Wq + np.asarray(fb1, f)  # (H, HID)
    for c in range(2):
        for h in range(H):
            b32[:, 7 + c * 4 + h] = hq[h, c * 128:(c + 1) * 128]
    base[:, _C_B32:_C_B32 + 30] = b32.view(f2).reshape(128, 30)
    base[:, _C_WQK:_C_WQK + 256] = Wq.astype(f2)
    base[:, _C_WQK + 256:_C_WQK + 512] = Wk.astype(f2)
    for c in range(2):
        base[:, _C_WZ + c * 63 + 31] = fw2[c * 128:(c + 1) * 128, 0].astype(f2)

    in_maps = []
    for k in range(NCORES):
        b0 = k * BL
        pk = base.copy()
        pk[0:32, _C_UAVT:_C_UAVT + BL * U] = \
            uav[b0:b0 + BL].reshape(BL * U, UAV_DIM).T.astype(f2)
        pk[0:32, _C_TASKT:_C_TASKT + BL * T] = \
            task[b0:b0 + BL].reshape(BL * T, TASK_DIM).T.astype(f2)
        in_maps.append({"packed": pk})
    return in_maps


def _gather(results):
    # out row p = 32*g + b*16 + h*4 + n4  with  n = 4*n4 + (g-h)%4,
    # cols = du*128 + t,  u = 4n + du.
    outs = []
    for k in range(NCORES):
        r = np.asarray(results[k]["out"], np.float32)  # (128, 512)
        o = np.empty((BL, H, U, T), np.float32)
        for g in range(4):
            for b in range(BL):
                for h in range(H):
                    for n4 in range(4):
                        p = 32 * g + b * 16 + h * 4 + n4
                        n = 4 * n4 + (g - h) % 4
                        row = r[p].reshape(4, T)      # (du, T)
                        o[b, h, 4 * n:4 * n + 4, :] = row
        outs.append(o)
    return np.concatenate(outs, axis=0)


def kernel(**inputs) -> np.ndarray:
    if "nc" not in _cache:
        _cache["nc"] = _build_nc()
    nc = _cache["nc"]
    in_maps = _prep_inputs(**inputs)
    if os.environ.get("BASS_KERNEL_SIM"):
        # CoreSim can't digest the hand-inserted wait-splitting NoOps; it
        # enforces the multi-wait semantics natively, so run unsplit.
        from concourse.bass_interp import CoreSim
        results = []
        for k in range(NCORES):
            sim = CoreSim(nc, require_nnan=False)
            for name, arr in in_maps[k].items():
                sim.tensor(name)[:] = arr
            sim.simulate()
            results.append({"out": np.array(sim.tensor("out"))})
    else:
        from concourse.bass_utils import run_bass_kernel_spmd
        if not _cache.get("split"):
            _split_multi_waits(nc)
            _cache["split"] = True
        results = run_bass_kernel_spmd(nc, in_maps, list(range(NCORES))).results
    return _gather(results)


# revision 37
# speedup vs baseline: 1.2072x; 1.0052x over previous
"""Trainium2 Bass kernel for nn_MultiHeadHighLevelAllocator.

Math (reference):
    ue = MLP3(uav_feat)                            # (B,U,E)
    te = MLP3(task_feat)                           # (B,T,E)
    q  = ue[:,None,:,:] + head_q[None,:,None,:]    # (B,H,U,E)
    logits[b,h,u,t] = relu(q[b,h,u]@Wq + te[b,t]@Wk + fb1) @ fw2 + fb2

Key decomposition: by linearity of the projections,
    pre[b,h,u,t,:] = base[b,u,t,:] + hqP[h,:]
where base[b,u,t,:] = ue[b,u]@Wq + te[b,t]@Wk  (outer sum, H-independent)
and   hqP[h,:] = head_q[h]@Wq + fb1.

Per core (data parallel over B, 2 batches/core), work is organized into
4096-column "units" (8 per core; cols = (u-block, t) pairs for one (b,c)
d-chunk).  Scheme per unit:

  B: TensorE builds base in PSUM via two accumulating broadcast-AP
     matmuls per 512 cols (Wq @ ue-bcast + Wk @ te-bcast), ScalarE
     copies PSUM->SBUF fp16 (FD=1024), VectorE does head relus as
     half-slab FD=2048 fused add+max ops (4x mode, started after two
     copies to halve the chain latency).  The unit's last bank is built
     by ScalarE directly (FD=128 khP + qP-bias ops) in its slack time,
     trimming two PE matmuls per unit; khP/qP projections are emitted
     lazily per section inside the pipeline.  `act_heads` optionally
     moves whole heads to ScalarE (unused in the final plan).
  A: VectorE builds base in SBUF fp16 (FD=128 tensor_scalar per u) from
     khP/qP projections, then long ADD,MAX ops per head (unused in the
     final plan).

The reduction against fw2 uses masked-stationary matmuls: a (128x32)
fp16 stationary holding the fw2-chunk in column j writes the dot product
row to PSUM partition 32g+j of strip g's own bank.  Strips cycle
round-robin (g = (n+h)%4) so up to 4 matmuls stream concurrently in
distinct PE column groups.  Reduction matmuls for unit k are emitted
after unit k+1's fills so the PE never idles waiting on the drain chain
(idle >3.4us re-throttles the PE clock to 1.2GHz).

A dummy-matmul warmup stream runs during the input DMA so the PE HAM
clock gate is at 2.4GHz before the encoders start.

All per-core inputs are packed host-side into ONE (128, 1824) fp16
tensor (weights/features fp16; biases bitcast fp32) loaded by
partition-split DMAs.
"""
import os
import sys

for _p in ("/opt/trn_rl_repo", "/root/.axon_site/_ro/trn_rl_repo"):
    if os.path.isdir(_p) and _p not in sys.path:
        sys.path.insert(0, _p)

import numpy as np
import concourse.bass as bass
import concourse.mybir as mybir
from concourse import tile

B, U, T = 16, 64, 128
UAV_DIM, TASK_DIM = 32, 32
E, H, HID = 128, 4, 256
ENC_H = 128
NCORES = 8
BL = B // NCORES          # batches per core
NBLK = U // 4             # 16 u-blocks of 4 us -> N=512 columns each
f32, f16 = mybir.dt.float32, mybir.dt.float16
AF = mybir.ActivationFunctionType
ALU = mybir.AluOpType

# packed tensor layout, fp16 columns
_C_UAVT = 0           # (32p, 128)
_C_TASKT = 128        # (32p, 256)
_C_UW0 = 384          # (32p, 128)
_C_TW0 = 512          # (32p, 128)
_C_B32 = 640          # 30 fp16 cols = 15 fp32: encb(7: ub0 ub1 ub2 tb0
#                       tb1 tb2 fb2) + hqpb(8: col c*4+h)
_C_UW1 = 672          # (128, 128) each
_C_UW2 = 800
_C_TW1 = 928
_C_TW2 = 1056
_C_WQK = 1184         # (128, 512): Wq c0 | Wq c1 | Wk c0 | Wk c1
# wz: two 63-col segments; fw2 chunk c at col c*63+31. The (128x32)
# masked stationary with fw2 at column j is the window [c*63+31-j, +32).
_C_WZ = 1696
_C_TOTAL = 1824

# number of N=128 dummy warmup matmuls issued before/through the encoder
_N_WARM = 20

# per-(b,c) section plan: scheme for each of the two 4096-col units,
# with the set of heads drained by ScalarE directly from PSUM ('B' only)
_PLAN = {
    (0, 0): (("B", ()), ("B", ())),
    (0, 1): (("B", ()), ("B", ())),
    (1, 0): (("B", ()), ("B", ())),
    (1, 1): (("B", ()), ("B", ())),
}

_cache: dict = {}


def _split_multi_waits(nc):
    """Walrus in this toolchain rejects >1 sync wait per engine instruction
    ("Too many sync wait commands"). Hoist extra waits onto preceding
    same-engine NoOps — identical semantics on the in-order engine queues."""
    n_split = 0
    for func in nc.m.functions:
        for bb in func.blocks:
            new = []
            for ins in bb.instructions:
                si = ins.sync_info
                waits = list(si.on_wait) if (si and si.on_wait) else []
                if len(waits) > 1:
                    for k, w in enumerate(waits[:-1]):
                        nop = mybir.InstNoOp(name=f"{ins.name}_hw{k}", ins=[], outs=[])
                        nop.engine = ins.engine
                        nop.sync_info = mybir.SyncInfo(on_wait=[w], on_update=[])
                        new.append(nop)
                        n_split += 1
                    si.on_wait = [waits[-1]]
                new.append(ins)
            bb.instructions = new
    return n_split


def _strip_map(b, h, n):
    """Round-robin strip assignment: strip g cycles 0..3 as (n+h) varies,
    j encodes (b, h, n//4). Bijective (b,h,n) <-> (g,j)."""
    g = (n + h) % 4
    j = b * 16 + h * 4 + n // 4
    return g, j


def _build_nc():
    nc = bass.Bass()
    packed = nc.dram_tensor("packed", [128, _C_TOTAL], f16, kind="ExternalInput")
    out = nc.dram_tensor("out", [128, 512], f32, kind="ExternalOutput")

    with tile.TileContext(nc) as tc:
        with (
            tc.tile_pool(name="const", bufs=1) as constp,
            tc.tile_pool(name="persist", bufs=1) as persistp,
            tc.tile_pool(name="encw", bufs=2) as encwp,
        ):
            A = constp.tile([128, _C_TOTAL], f16, tag="all")
            # phase-ordered, partition-split loads: L1 inputs + biases
            # first so the encoders start as early as possible
            for p0 in (0, 64):
                nc.sync.dma_start(A[p0:p0 + 64, :_C_UW1],
                                  packed[p0:p0 + 64, :_C_UW1])
            for p0 in (0, 64):
                nc.sync.dma_start(A[p0:p0 + 64, _C_UW1:_C_WQK],
                                  packed[p0:p0 + 64, _C_UW1:_C_WQK])
            for p0 in (0, 64):
                nc.sync.dma_start(A[p0:p0 + 64, _C_WQK:],
                                  packed[p0:p0 + 64, _C_WQK:])
            bc32 = A[:, _C_B32:_C_B32 + 30].bitcast(f32)
            sb_wz = A[:, _C_WZ:_C_WZ + 126]
            sb_wqk16 = A[:, _C_WQK:_C_WQK + 512]

            enc_w = {
                "uw0": A[0:32, _C_UW0:_C_UW0 + 128],
                "tw0": A[0:32, _C_TW0:_C_TW0 + 128],
                "uw1": A[:, _C_UW1:_C_UW1 + 128],
                "uw2": A[:, _C_UW2:_C_UW2 + 128],
                "tw1": A[:, _C_TW1:_C_TW1 + 128],
                "tw2": A[:, _C_TW2:_C_TW2 + 128],
            }

            def encb_col(i):
                return bc32[:, i:i + 1]

            def hqp_col(c, h):
                return bc32[:, 7 + c * 4 + h:7 + c * 4 + h + 1]

            # ---- encoder phase: own PSUM pools, released before sections ----
            with (
                tc.tile_pool(name="warmp", bufs=1, space="PSUM") as warmp,
                tc.tile_pool(name="encp", bufs=4, space="PSUM") as psE,
            ):
                # PE warmup stream: runs on memset data with no DMA
                # dependency, keeping the PE busy through the input DMA so
                # HAM is at K=8/8 when real matmuls start.  Encoder matmuls
                # are interleaved into the stream so the encoder pipeline
                # overlaps the warmup instead of queueing behind it.
                wsrc = constp.tile([128, 8], f16, tag="wsrc")
                nc.vector.memset(wsrc[:], 0.0)
                wps = warmp.tile([8, 512], f32, tag="warm")
                wmov = wsrc[:, 0:1].broadcast_to([128, 128])

                def warm(n):
                    for _ in range(n):
                        nc.tensor.matmul(wps[:, :128], wsrc[:], wmov,
                                         start=True, stop=True)

                warm(_N_WARM)
                chains = {  # noqa: encoder matmuls interleave with warm()
                    "ue": [A[0:32, _C_UAVT:_C_UAVT + BL * U], BL * U,
                           ("uw0", "uw1", "uw2"), (0, 1, 2)],
                    "te": [A[0:32, _C_TASKT:_C_TASKT + BL * T], BL * T,
                           ("tw0", "tw1", "tw2"), (3, 4, 5)],
                }
                cur = {k: v[0] for k, v in chains.items()}
                for li in range(3):
                    pss = {}
                    for k, (x0, rows, wn, bc) in chains.items():
                        ps = psE.tile([128, 512], f32, tag="ep",
                                      name=f"ps{k}{li}")
                        nc.tensor.matmul(ps[:, :rows], enc_w[wn[li]], cur[k],
                                         start=True, stop=True)
                        pss[k] = ps
                    warm(7)  # keep the PE busy while ScalarE runs the relus
                    for k, (x0, rows, wn, bc) in chains.items():
                        if li < 2:
                            nxt = encwp.tile([128, rows], f16, tag=f"{k}h",
                                             name=f"{k}h{li}")
                            nc.scalar.activation(nxt[:], pss[k][:, :rows],
                                                 AF.Relu,
                                                 bias=encb_col(bc[li]),
                                                 scale=1.0)
                        else:
                            nxt = persistp.tile([128, rows], f16, tag=f"{k}T",
                                                name=f"{k}T")
                            nc.scalar.activation(nxt[:], pss[k][:, :rows],
                                                 AF.Identity,
                                                 bias=encb_col(bc[li]),
                                                 scale=1.0)
                        cur[k] = nxt[:]
                ueT, teT = cur["ue"], cur["te"]

                khPs, qPs = {}, {}

            # ---- section phase ----
            with (
                tc.tile_pool(name="bsbp", bufs=4) as bsbp,
                tc.tile_pool(name="relup", bufs=10) as relup,
                tc.tile_pool(name="outp", bufs=1) as outp,
                tc.tile_pool(name="psSp", bufs=2, space="PSUM") as psS,
                tc.tile_pool(name="lpp", bufs=1, space="PSUM") as psL,
            ):
                lp = [psL.tile([128, 512], f32, tag=f"lp{g}", name=f"lp{g}")
                      for g in range(4)]
                n_red = [0, 0, 0, 0]
                N_RED_PER_STRIP = BL * 2 * NBLK * H // 4

                def red_mm(b, c, h, n, rt_ap):
                    """One masked-stationary reduction matmul: 512 cols of
                    rt for (b,c,h,u-block n) -> partition 32g+j of lp[g]."""
                    g, j = _strip_map(b, h, n)
                    first = n_red[g] == 0
                    last = n_red[g] == N_RED_PER_STRIP - 1
                    n_red[g] += 1
                    nc.tensor.matmul(
                        lp[g][32 * g:32 * g + 32, :],
                        sb_wz[:, c * 63 + 31 - j:c * 63 + 63 - j],
                        rt_ap,
                        start=first, stop=last,
                        tile_position=(0, 32 * g),
                        skip_group_check=True)

                def pe_base_fill(b, c, n0, ps_tile):
                    """Fill a [128,1024] PSUM tile (2 banks) with base cols
                    for u-blocks n0, n0+1 via broadcast-AP matmuls.  Both
                    Wq matmuls are issued back-to-back (then both Wk) so
                    the second LDWEIGHTS of each pair is a near-free
                    same-stationary reload."""
                    wq = sb_wqk16[:, c * 128:(c + 1) * 128]
                    wk = sb_wqk16[:, 256 + c * 128:256 + (c + 1) * 128]
                    tem = teT[:, b * T:(b + 1) * T].unsqueeze(1) \
                        .broadcast_to([128, 4, T])
                    for dn in range(2):
                        n = n0 + dn
                        uem = ueT[:, b * U + 4 * n:b * U + 4 * n + 4] \
                            .unsqueeze(2).broadcast_to([128, 4, T])
                        dst = ps_tile[:, dn * 512:(dn + 1) * 512]
                        nc.tensor.matmul(dst, wq, uem, start=True, stop=False)
                    for dn in range(2):
                        dst = ps_tile[:, dn * 512:(dn + 1) * 512]
                        nc.tensor.matmul(dst, wk, tem, start=False, stop=True)

                def prelude(b, c):
                    """Lazy per-section khP/qP projections, emitted inside
                    the pipeline (PSUM borrowed from the fill pool)."""
                    if (b, c) in khPs:
                        return
                    pp = psS.tile([128, 1024], f32, tag="ps",
                                  name=f"pp{b}{c}")
                    nc.tensor.matmul(pp[:, :T],
                                     sb_wqk16[:, 256 + c * 128:256 + (c + 1) * 128],
                                     teT[:, b * T:(b + 1) * T],
                                     start=True, stop=True)
                    nc.tensor.matmul(pp[:, 512:512 + U],
                                     sb_wqk16[:, c * 128:(c + 1) * 128],
                                     ueT[:, b * U:(b + 1) * U],
                                     start=True, stop=True)
                    khP = persistp.tile([128, T], f16, tag=f"khP{b}{c}",
                                        name=f"khP{b}{c}")
                    nc.scalar.copy(khP[:], pp[:, :T])
                    qP = persistp.tile([128, U], f32, tag=f"qP{b}{c}",
                                       name=f"qP{b}{c}")
                    nc.scalar.copy(qP[:], pp[:, 512:512 + U])
                    khPs[(b, c)], qPs[(b, c)] = khP, qP

                def pe_base_fill_single(b, c, n, dst):
                    wq = sb_wqk16[:, c * 128:(c + 1) * 128]
                    wk = sb_wqk16[:, 256 + c * 128:256 + (c + 1) * 128]
                    tem = teT[:, b * T:(b + 1) * T].unsqueeze(1) \
                        .broadcast_to([128, 4, T])
                    uem = ueT[:, b * U + 4 * n:b * U + 4 * n + 4] \
                        .unsqueeze(2).broadcast_to([128, 4, T])
                    nc.tensor.matmul(dst, wq, uem, start=True, stop=False)
                    nc.tensor.matmul(dst, wk, tem, start=False, stop=True)

                def emit_unit(b, c, n0, scheme, act_heads, fine=False):
                    """Emit fills + drains for one 4096-col unit
                    (u-blocks n0..n0+7); returns the per-head rt tiles."""
                    if scheme == "A":
                        khP, qP = khPs[(b, c)], qPs[(b, c)]
                        bsb = bsbp.tile([128, 4096], f16, tag="bsb",
                                        name="bsb")
                        for dn in range(8):
                            for du in range(4):
                                u = 4 * (n0 + dn) + du
                                nc.vector.tensor_scalar(
                                    bsb[:, dn * 512 + du * 128:
                                        dn * 512 + (du + 1) * 128],
                                    khP[:], qP[:, u:u + 1], None, ALU.add)
                        rts = {}
                        for h in range(H):
                            rt = relup.tile([128, 4096], f16, tag="rt",
                                            name="rt")
                            nc.vector.tensor_scalar(
                                rt[:], bsb[:], hqp_col(c, h), 0.0,
                                ALU.add, ALU.max)
                            rts[h] = rt
                        return rts

                    assert scheme == "B"
                    dve_heads = [h for h in range(H) if h not in act_heads]
                    rts = {h: relup.tile([128, 4096], f16, tag="rt",
                                         name="rt") for h in range(H)}
                    bsb = None
                    if dve_heads:
                        bsb = bsbp.tile([128, 4096], f16, tag="bsb",
                                        name="bsb")
                    prelude(b, c)
                    khP, qP = khPs[(b, c)], qPs[(b, c)]
                    for f in range(4):
                        ps_tile = psS.tile([128, 1024], f32, tag="ps",
                                           name=f"ps{b}{c}{n0}{f}")
                        if f < 3:
                            pe_base_fill(b, c, n0 + 2 * f, ps_tile)
                            nc.scalar.copy(bsb[:, f * 1024:(f + 1) * 1024],
                                           ps_tile[:])
                        else:
                            # bank 6 via the PE; bank 7 is built by ScalarE
                            # (FD=128 khP+qP bias ops) in its slack time,
                            # trimming two PE matmuls per unit
                            pe_base_fill_single(b, c, n0 + 6,
                                                ps_tile[:, 0:512])
                            for du in range(4):
                                u = 4 * (n0 + 7) + du
                                nc.scalar.activation(
                                    bsb[:, 3584 + du * 128:
                                        3584 + (du + 1) * 128],
                                    khP[:], AF.Identity,
                                    bias=qP[:, u:u + 1], scale=1.0)
                            nc.scalar.copy(bsb[:, 3072:3584],
                                           ps_tile[:, 0:512])
                        # half-slab relus (FD=2048): halves the copy->relu
                        # chain latency so reduction matmuls never stall.
                        # The last unit drains its second half per-fill
                        # (FD=1024) so the pipeline tail is shorter.
                        if fine and f in (2, 3):
                            lo = f * 1024
                            for h in dve_heads:
                                nc.vector.tensor_scalar(
                                    rts[h][:, lo:lo + 1024],
                                    bsb[:, lo:lo + 1024], hqp_col(c, h), 0.0,
                                    ALU.add, ALU.max)
                        elif f == 1 or (f == 3 and not fine):
                            lo = (f - 1) * 1024
                            for h in dve_heads:
                                nc.vector.tensor_scalar(
                                    rts[h][:, lo:lo + 2048],
                                    bsb[:, lo:lo + 2048], hqp_col(c, h), 0.0,
                                    ALU.add, ALU.max)
                    return rts

                units = []
                for b in range(BL):
                    for c in range(2):
                        for ui, (scheme, ah) in enumerate(_PLAN[(b, c)]):
                            units.append((b, c, 8 * ui, scheme, ah))

                pending = None  # (b, c, n0, rts) awaiting reduction
                for ui, (b, c, n0, scheme, ah) in enumerate(units):
                    rts = emit_unit(b, c, n0, scheme, ah,
                                    fine=(ui == len(units) - 1))
                    if pending is not None:
                        pb, pc, pn0, prts = pending
                        for h in range(H):
                            for dn in range(8):
                                red_mm(pb, pc, h, pn0 + dn,
                                       prts[h][:, dn * 512:(dn + 1) * 512])
                    pending = (b, c, n0, rts)
                pb, pc, pn0, prts = pending
                for h in range(H):
                    for dn in range(8):
                        red_mm(pb, pc, h, pn0 + dn,
                               prts[h][:, dn * 512:(dn + 1) * 512])

                # per-strip epilogue split across ScalarE and VectorE (both
                # idle at the tail), with a split DMA so each strip ships
                # as soon as it is ready
                sb_out = outp.tile([128, 512], f32, tag="sbout", name="sbout")
                for g in range(4):
                    if g % 2 == 0:
                        nc.scalar.activation(
                            sb_out[32 * g:32 * g + 32, :],
                            lp[g][32 * g:32 * g + 32, :],
                            AF.Identity, bias=bc32[32 * g:32 * g + 32, 6:7],
                            scale=1.0)
                    else:
                        nc.vector.tensor_scalar(
                            sb_out[32 * g:32 * g + 32, :],
                            lp[g][32 * g:32 * g + 32, :],
                            bc32[32 * g:32 * g + 32, 6:7],
                            None, ALU.add)
                    nc.sync.dma_start(out[32 * g:32 * g + 32, :],
                                      sb_out[32 * g:32 * g + 32, :])
    return nc


def _prep_inputs(uav_feat, task_feat, uw0, ub0, uw1, ub1, uw2, ub2,
                 tw0, tb0, tw1, tb1, tw2, tb2, head_q, fw1, fb1, fw2, fb2):
    f, f2 = np.float32, np.float16
    uav = np.asarray(uav_feat, f)
    task = np.asarray(task_feat, f)
    fw1 = np.asarray(fw1, f)
    fw2 = np.asarray(fw2, f)
    Wq, Wk = fw1[:E], fw1[E:]

    base = np.zeros((128, _C_TOTAL), f2)
    base[0:32, _C_UW0:_C_UW0 + 128] = np.asarray(uw0, f2)
    base[0:32, _C_TW0:_C_TW0 + 128] = np.asarray(tw0, f2)
    base[:, _C_UW1:_C_UW1 + 128] = np.asarray(uw1, f2)
    base[:, _C_UW2:_C_UW2 + 128] = np.asarray(uw2, f2)
    base[:, _C_TW1:_C_TW1 + 128] = np.asarray(tw1, f2)
    base[:, _C_TW2:_C_TW2 + 128] = np.asarray(tw2, f2)
    # fp32 bias region, bitcast into the fp16 tensor
    b32 = np.zeros((128, 15), f)
    for i, v in enumerate((ub0, ub1, ub2, tb0, tb1, tb2)):
        b32[:, i] = np.asarray(v, f)
    b32[:, 6] = np.asarray(fb2, f)[0]
    hq = np.asarray(head_q, f) @ Wq + np.asarray(fb1, f)  # (H, HID)
    for c in range(2):
        for h in range(H):
            b32[:, 7 + c * 4 + h] = hq[h, c * 128:(c + 1) * 128]
    base[:, _C_B32:_C_B32 + 30] = b32.view(f2).reshape(128, 30)
    base[:, _C_WQK:_C_WQK + 256] = Wq.astype(f2)
    base[:, _C_WQK + 256:_C_WQK + 512] = Wk.astype(f2)
    for c in range(2):
        base[:, _C_WZ + c * 63 + 31] = fw2[c * 128:(c + 1) * 128, 0].astype(f2)

    in_maps = []
    for k in range(NCORES):
        b0 = k * BL
        pk = base.copy()
        pk[0:32, _C_UAVT:_C_UAVT + BL * U] = \
            uav[b0:b0 + BL].reshape(BL * U, UAV_DIM).T.astype(f2)
        pk[0:32, _C_TASKT:_C_TASKT + BL * T] = \
            task[b0:b0 + BL].reshape(BL * T, TASK_DIM).T.astype(f2)
        in_maps.append({"packed": pk})
    return in_maps


def _gather(results):
    # out row p = 32*g + b*16 + h*4 + n4  with  n = 4*n4 + (g-h)%4,
    # cols = du*128 + t,  u = 4n + du.
    outs = []
    for k in range(NCORES):
        r = np.asarray(results[k]["out"], np.float32)  # (128, 512)
        o = np.empty((BL, H, U, T), np.float32)
        for g in range(4):
            for b in range(BL):
                for h in range(H):
                    for n4 in range(4):
                        p = 32 * g + b * 16 + h * 4 + n4
                        n = 4 * n4 + (g - h) % 4
                        row = r[p].reshape(4, T)      # (du, T)
                        o[b, h, 4 * n:4 * n + 4, :] = row
        outs.append(o)
    return np.concatenate(outs, axis=0)


def kernel(**inputs) -> np.ndarray:
    if "nc" not in _cache:
        _cache["nc"] = _build_nc()
    nc = _cache["nc"]
    in_maps = _prep_inputs(**inputs)
    if os.environ.get("BASS_KERNEL_SIM"):
        # CoreSim can't digest the hand-inserted wait-splitting NoOps; it
        # enforces the multi-wait semantics natively, so run unsplit.
        from concourse.bass_interp import CoreSim
        results = []
        for k in range(NCORES):
            sim = CoreSim(nc, require_nnan=False)
            for name, arr in in_maps[k].items():
                sim.tensor(name)[:] = arr
            sim.simulate()
            results.append({"out": np.array(sim.tensor("out"))})
    else:
        from concourse.bass_utils import run_bass_kernel_spmd
        if not _cache.get("split"):
            _split_multi_waits(nc)
            _cache["split"] = True
        results = run_bass_kernel_spmd(nc, in_maps, list(range(NCORES))).results
    return _gather(results)
